# revision 7
# baseline (speedup 1.0000x reference)
"""Distributed Trainium2 (Bass) kernel for additive (Bahdanau) attention.

Strategy
--------
The reference computes  scores[b,i,j] = sum_d v[d] * tanh(qp[b,i,d] + kp[b,j,d])
which is O(B*Lq*Lk*Dk) transcendental work (134M tanh) if done directly.
We replace tanh with a 6-term sine expansion

    tanh(s) ~= sum_m c_m sin(w_m s)            (fit on s in [-7.2, 7.2])

and use  sin(w(a+b)) = sin(wa)cos(wb) + cos(wa)sin(wb)  to turn the score
tensor into a single fp32r matmul over 768 "trig feature" rows:

    scoresT = sum_m Qf_m^T @ Pf_m
      Qf_m = [cos(w_m kpT); sin(w_m kpT)]            [128, Lk]
      Pf_m = [sin(w_m qpT); cos(w_m qpT)] * (c_m v)  [128, Lq_shard]

ACT's Sin is only valid on ~[-3.4, 3.4], so arguments are range-reduced on
DVE with the fused magic-number round trick (one tensor_scalar per pass).

Sharding: 8 cores = 2 batches x 4 query-blocks of 256 rows.  K/V (transposed
on host, bf16) are replicated per batch; everything else is local, so there
are no collectives.  attn is produced transposed ([Lk, 256] per core) and
transposed back on the host during unsharding.
"""

import sys
import numpy as np

if "/opt/trn_rl_repo" not in sys.path:
    sys.path.insert(0, "/opt/trn_rl_repo")

import ml_dtypes

B, LQ, LK, DM, DK, DV = 2, 1024, 1024, 512, 64, 64
QS = 256          # query rows per core
NCORES = 8
M = 6             # sine terms
LN_EPS = 1e-6
MAGIC = 12582912.0  # 1.5 * 2^23: fused (x+MAGIC)-MAGIC == round(x) on DVE

# tanh(s) ~= sum c_m sin(om_m s), weighted LSQ fit on [-7.2, 7.2]
CS = np.array([1.212018, 0.277867, 0.085554, 0.007496, 0.025805, 0.00188])
OMS = np.array([0.358669, 1.08828, 1.847415, 3.511154, 2.644391, 4.567425])
OMP = OMS / (2 * np.pi)

_CACHE = {}


def _build(trace=False):
    """Build (and cache) the Bass graph."""
    if "nc" in _CACHE:
        return _CACHE["nc"]
    import concourse.bacc as bacc
    import concourse.mybir as mybir
    from concourse.tile import TileContext

    f32 = mybir.dt.float32
    f32r = mybir.dt.float32r
    bf16 = mybir.dt.bfloat16
    AF = mybir.ActivationFunctionType
    ALU = mybir.AluOpType

    nc = bacc.Bacc("TRN2", target_bir_lowering=False, debug=False,
                   num_devices=NCORES)

    qT_d = nc.dram_tensor("qT", [DM, QS], bf16, kind="ExternalInput").ap()
    qn_d = nc.dram_tensor("qn", [QS, DM], f32, kind="ExternalInput").ap()
    kT_d = nc.dram_tensor("kT", [DM, LK], bf16, kind="ExternalInput").ap()
    vT_d = nc.dram_tensor("vT", [DM, LK], bf16, kind="ExternalInput").ap()
    wq2_d = nc.dram_tensor("wq2", [DM, 128], bf16, kind="ExternalInput").ap()
    wk2_d = nc.dram_tensor("wk2", [DM, 128], bf16, kind="ExternalInput").ap()
    wv_d = nc.dram_tensor("wv", [DM, DV], bf16, kind="ExternalInput").ap()
    wfc_d = nc.dram_tensor("wfc", [DV, DM], f32, kind="ExternalInput").ap()
    pco_d = nc.dram_tensor("pco", [128, M], f32, kind="ExternalInput").ap()
    gam_d = nc.dram_tensor("gam", [1, DM], f32, kind="ExternalInput").ap()
    bet_d = nc.dram_tensor("bet", [1, DM], f32, kind="ExternalInput").ap()
    eye_d = nc.dram_tensor("eye", [64, 64], f32, kind="ExternalInput").ap()

    attn_d = nc.dram_tensor("attnT", [LK, QS], f32, kind="ExternalOutput").ap()
    out_d = nc.dram_tensor("outp", [QS, DM], f32, kind="ExternalOutput").ap()

    import concourse.bass as bass  # noqa: F401

    with TileContext(nc) as tc:
        import contextlib
        with contextlib.ExitStack() as ctx:
            const = ctx.enter_context(tc.tile_pool(name="const", bufs=1))
            work = ctx.enter_context(tc.tile_pool(name="work", bufs=1))
            small = ctx.enter_context(tc.tile_pool(name="small", bufs=2))
            ps1 = ctx.enter_context(tc.tile_pool(name="ps1", bufs=1, space="PSUM"))
            ps2 = ctx.enter_context(tc.tile_pool(name="ps2", bufs=2, space="PSUM"))
            ps3 = ctx.enter_context(tc.tile_pool(name="ps3", bufs=1, space="PSUM"))

            # ---------------- constants / inputs -> SBUF ----------------
            kT_sb = const.tile([128, 4, LK], bf16)     # [Dm-tile, lk]
            nc.sync.dma_start(out=kT_sb, in_=kT_d.rearrange("(t p) l -> p t l", p=128))
            vT_sb = const.tile([128, 4, LK], bf16)
            nc.sync.dma_start(out=vT_sb, in_=vT_d.rearrange("(t p) l -> p t l", p=128))
            qT_sb = const.tile([128, 4, QS], bf16)
            nc.sync.dma_start(out=qT_sb, in_=qT_d.rearrange("(t p) l -> p t l", p=128))
            qn_sb = const.tile([128, 2, DM], f32)
            nc.sync.dma_start(out=qn_sb, in_=qn_d.rearrange("(t p) l -> p t l", p=128))
            wq2_sb = const.tile([128, 4, 128], bf16)
            nc.sync.dma_start(out=wq2_sb, in_=wq2_d.rearrange("(t p) m -> p t m", p=128))
            wk2_sb = const.tile([128, 4, 128], bf16)
            nc.sync.dma_start(out=wk2_sb, in_=wk2_d.rearrange("(t p) m -> p t m", p=128))
            wv_sb = const.tile([128, 4, DV], bf16)
            nc.sync.dma_start(out=wv_sb, in_=wv_d.rearrange("(t p) m -> p t m", p=128))
            wfc_f = const.tile([64, DM], f32)
            nc.sync.dma_start(out=wfc_f, in_=wfc_d)
            pco_sb = const.tile([128, M], f32)
            nc.sync.dma_start(out=pco_sb, in_=pco_d)
            eye_sb = const.tile([64, 64], f32)
            nc.sync.dma_start(out=eye_sb, in_=eye_d)
            gam_sb = const.tile([128, DM], f32)
            gam_b = type(gam_d)(tensor=gam_d.tensor, offset=gam_d.offset,
                                ap=[[0, 128]] + gam_d.ap[1:])
            nc.sync.dma_start(out=gam_sb, in_=gam_b)
            bet_sb = const.tile([128, DM], f32)
            bet_b = type(bet_d)(tensor=bet_d.tensor, offset=bet_d.offset,
                                ap=[[0, 128]] + bet_d.ap[1:])
            nc.sync.dma_start(out=bet_sb, in_=bet_b)

            wfc_r = const.tile([64, DM], f32r)
            nc.vector.tensor_copy(wfc_r[:], wfc_f[:])

            dvQ = const.tile([128, 1], f32)   # cos rows then sin rows
            nc.vector.memset(dvQ[0:64, :], 0.25)
            nc.vector.memset(dvQ[64:128, :], 0.0)
            dvP = const.tile([128, 1], f32)   # sin rows then cos rows
            nc.vector.memset(dvP[0:64, :], 0.0)
            nc.vector.memset(dvP[64:128, :], 0.25)
            ones_f = const.tile([128, 1], f32)
            nc.vector.memset(ones_f, 1.0)
            ones_r = const.tile([128, 1], f32r)
            nc.vector.tensor_copy(ones_r[:], ones_f[:])
            ones1_f = const.tile([1, 128], f32)
            nc.vector.memset(ones1_f, 1.0)
            ones1_r = const.tile([1, 128], f32r)
            nc.vector.tensor_copy(ones1_r[:], ones1_f[:])
            eps_sb = const.tile([128, 1], f32)
            nc.vector.memset(eps_sb, LN_EPS)

            # ---------------- projections (bf16 matmuls) ----------------
            kp2_ps = ps1.tile([128, LK], f32, tag="kq")         # [dup 128, lk] 2 banks
            for t in range(4):
                for h in range(2):
                    nc.tensor.matmul(kp2_ps[:, h * 512:(h + 1) * 512],
                                     wk2_sb[:, t, :],
                                     kT_sb[:, t, h * 512:(h + 1) * 512],
                                     start=(t == 0), stop=(t == 3))
            qp2_ps = ps1.tile([128, QS], f32, tag="qp")
            for t in range(4):
                nc.tensor.matmul(qp2_ps[:], wq2_sb[:, t, :], qT_sb[:, t, :],
                                 start=(t == 0), stop=(t == 3))
            # ---------------- trig features -----------------------------
            # Q side: u = kpT2*om' + dvQ ; n = round(u) ; r = u - n ; sin(2pi r)
            uQ = work.tile([128, M, LK], f32)
            for m in range(M):
                nc.vector.tensor_scalar(out=uQ[:, m, :], in0=kp2_ps[:],
                                        scalar1=float(OMP[m]), scalar2=dvQ[:],
                                        op0=ALU.mult, op1=ALU.add)
            vp_ps = ps1.tile([64, LK], f32, tag="kq")
            for t in range(4):
                for h in range(2):
                    nc.tensor.matmul(vp_ps[:, h * 512:(h + 1) * 512],
                                     wv_sb[:, t, :],
                                     vT_sb[:, t, h * 512:(h + 1) * 512],
                                     start=(t == 0), stop=(t == 3))

            nQ = work.tile([128, M, LK], f32)
            nc.vector.tensor_scalar(out=nQ[:], in0=uQ[:], scalar1=MAGIC,
                                    scalar2=-MAGIC, op0=ALU.add, op1=ALU.add)
            rQ = work.tile([128, M, LK], f32)
            nc.gpsimd.tensor_tensor(out=rQ[:], in0=uQ[:], in1=nQ[:],
                                    op=ALU.subtract)
            Qf = work.tile([128, M, LK], f32r)
            nc.scalar.activation(out=Qf[:], in_=rQ[:], func=AF.Sin,
                                 scale=float(2 * np.pi))

            # P side (also multiplied by c_m * v_param afterwards)
            uP = work.tile([128, M, QS], f32)
            for m in range(M):
                nc.vector.tensor_scalar(out=uP[:, m, :], in0=qp2_ps[:],
                                        scalar1=float(OMP[m]), scalar2=dvP[:],
                                        op0=ALU.mult, op1=ALU.add)
            nP = work.tile([128, M, QS], f32)
            nc.vector.tensor_scalar(out=nP[:], in0=uP[:], scalar1=MAGIC,
                                    scalar2=-MAGIC, op0=ALU.add, op1=ALU.add)
            rP = work.tile([128, M, QS], f32)
            nc.gpsimd.tensor_tensor(out=rP[:], in0=uP[:], in1=nP[:],
                                    op=ALU.subtract)
            Pf_raw = work.tile([128, M, QS], f32)
            nc.scalar.activation(out=Pf_raw[:], in_=rP[:], func=AF.Sin,
                                 scale=float(2 * np.pi))
            Pf = work.tile([128, M, QS], f32r)
            for m in range(M):
                nc.vector.tensor_scalar(out=Pf[:, m, :], in0=Pf_raw[:, m, :],
                                        scalar1=pco_sb[:, m:m + 1], scalar2=None,
                                        op0=ALU.mult)

            # -------- vp transpose: vpT [64, LK] -> vp [LK-tiles, 64] ----
            vpT_sb = work.tile([64, LK], f32)
            nc.vector.tensor_copy(vpT_sb[:], vp_ps[:])
            vp_sb = work.tile([128, 8, 64], f32r)
            tr_ps = ps3.tile([128, 8, 64], f32, tag="tr")
            for kc in range(8):
                nc.tensor.transpose(tr_ps[:, kc, :], vpT_sb[:, kc * 128:(kc + 1) * 128],
                                    eye_sb[:])
            nc.vector.tensor_copy(vp_sb[:], tr_ps[:])

            # ---------------- scores / softmax / av ---------------------
            den_ps = ps3.tile([1, QS], f32, tag="db")
            uT_ps = ps3.tile([64, QS], f32, tag="uT")
            expT = work.tile([128, 8, QS], f32r)
            for kc in range(8):
                sc_full = ps2.tile([128, DM], f32, tag="sc")
                sc_ps = sc_full[:, 0:QS]
                for m in range(M):
                    nc.tensor.matmul(
                        sc_ps[:],
                        Qf[:, m, kc * 128:(kc + 1) * 128],
                        Pf[:, m, :],
                        start=(m == 0), stop=(m == M - 1))
                nc.scalar.activation(out=expT[:, kc, :], in_=sc_ps[:],
                                     func=AF.Exp, scale=1.0)
                nc.tensor.matmul(den_ps[:], ones_r[:], expT[:, kc, :],
                                 start=(kc == 0), stop=(kc == 7))
                nc.tensor.matmul(uT_ps[:], vp_sb[:, kc, :], expT[:, kc, :],
                                 start=(kc == 0), stop=(kc == 7))

            rec_f = small.tile([1, QS], f32)
            nc.vector.reciprocal(out=rec_f[:], in_=den_ps[:])
            rec_r = small.tile([1, QS], f32r)
            nc.vector.tensor_copy(rec_r[:], rec_f[:])
            bc_ps = ps3.tile([128, QS], f32, tag="db")
            nc.tensor.matmul(bc_ps[:], ones1_r[:], rec_r[:], start=True, stop=True)
            bc_sb = work.tile([128, QS], f32)
            nc.vector.tensor_copy(bc_sb[:], bc_ps[:])

            # attn output (normalized), written transposed
            for kc in range(8):
                at_sb = small.tile([128, QS], f32, tag="at")
                nc.gpsimd.tensor_tensor(out=at_sb[:],
                                        in0=expT[:, kc, :].bitcast(f32),
                                        in1=bc_sb[:], op=ALU.mult)
                nc.sync.dma_start(out=attn_d[kc * 128:(kc + 1) * 128, :],
                                  in_=at_sb[:])

            # normalized attention-value product
            avn = work.tile([64, QS], f32r)
            nc.vector.tensor_tensor(out=avn[:], in0=uT_ps[:], in1=bc_sb[0:64, :],
                                    op=ALU.mult)

            # ---------------- fc + residual + layernorm -----------------
            for qc in range(2):
                o_ps = ps2.tile([128, DM], f32, tag="sc")
                nc.tensor.matmul(o_ps[:], avn[:, qc * 128:(qc + 1) * 128],
                                 wfc_r[:], start=True, stop=True)
                o1 = small.tile([128, DM], f32, tag="o1")
                nc.vector.tensor_tensor(out=o1[:], in0=o_ps[:],
                                        in1=qn_sb[:, qc, :], op=ALU.add)
                st = small.tile([128, 6], f32, tag="st")
                nc.vector.bn_stats(out=st[:], in_=o1[:])
                mv = small.tile([128, 2], f32, tag="mv")
                nc.vector.bn_aggr(out=mv[:], in_=st[:])
                sq = small.tile([128, 1], f32, tag="sq")
                nc.scalar.activation(out=sq[:], in_=mv[:, 1:2], func=AF.Sqrt,
                                     scale=1.0, bias=eps_sb[:])
                rs = small.tile([128, 1], f32, tag="rs")
                nc.vector.reciprocal(out=rs[:], in_=sq[:])
                o2 = small.tile([128, DM], f32, tag="o2")
                nc.vector.tensor_scalar(out=o2[:], in0=o1[:],
                                        scalar1=mv[:, 0:1], scalar2=rs[:],
                                        op0=ALU.subtract, op1=ALU.mult)
                o3 = small.tile([128, DM], f32, tag="o3")
                nc.gpsimd.tensor_tensor(out=o3[:], in0=o2[:], in1=gam_sb[:],
                                        op=ALU.mult)
                o4 = small.tile([128, DM], f32, tag="o4")
                nc.gpsimd.tensor_tensor(out=o4[:], in0=o3[:], in1=bet_sb[:],
                                        op=ALU.add)
                nc.sync.dma_start(
                    out=out_d.rearrange("(t p) l -> p t l", p=128)[:, qc, :],
                    in_=o4[:])

    nc.finalize()
    _CACHE["nc"] = nc
    return nc


def _prep_inputs(q, k, v, Wq, Wk, Wv, v_param, Wfc, ln_gamma, ln_beta):
    bf = ml_dtypes.bfloat16
    wq2 = np.ascontiguousarray(np.concatenate([Wq, Wq], axis=1)).astype(bf)
    wk2 = np.ascontiguousarray(np.concatenate([Wk, Wk], axis=1)).astype(bf)
    wv = np.ascontiguousarray(Wv).astype(bf)
    wfc = np.ascontiguousarray(Wfc).astype(np.float32)
    pco = np.zeros((128, M), dtype=np.float32)
    for m in range(M):
        pco[0:64, m] = (CS[m] * v_param).astype(np.float32)
        pco[64:128, m] = (CS[m] * v_param).astype(np.float32)
    gam = np.ascontiguousarray(ln_gamma.reshape(1, DM)).astype(np.float32)
    bet = np.ascontiguousarray(ln_beta.reshape(1, DM)).astype(np.float32)
    eye = np.eye(64, dtype=np.float32)

    kT = [np.ascontiguousarray(k[b].T).astype(bf) for b in range(B)]
    vT = [np.ascontiguousarray(v[b].T).astype(bf) for b in range(B)]

    in_maps = []
    for core in range(NCORES):
        b, q0 = core // 4, QS * (core % 4)
        qs = q[b, q0:q0 + QS]
        in_maps.append({
            "qT": np.ascontiguousarray(qs.T).astype(bf),
            "qn": np.ascontiguousarray(qs).astype(np.float32),
            "kT": kT[b],
            "vT": vT[b],
            "wq2": wq2, "wk2": wk2, "wv": wv, "wfc": wfc,
            "pco": pco, "gam": gam, "bet": bet, "eye": eye,
        })
    return in_maps


def kernel(q, k, v, Wq, Wk, Wv, v_param, Wfc, ln_gamma, ln_beta,
           _trace=False):
    q = np.asarray(q); k = np.asarray(k); v = np.asarray(v)
    Wq = np.asarray(Wq); Wk = np.asarray(Wk); Wv = np.asarray(Wv)
    v_param = np.asarray(v_param); Wfc = np.asarray(Wfc)
    ln_gamma = np.asarray(ln_gamma); ln_beta = np.asarray(ln_beta)

    from concourse.bass_utils import run_bass_kernel_spmd

    nc = _build()
    in_maps = _prep_inputs(q, k, v, Wq, Wk, Wv, v_param, Wfc, ln_gamma, ln_beta)
    res = run_bass_kernel_spmd(nc, in_maps, core_ids=list(range(NCORES)),
                               trace=_trace)

    out = np.zeros((B, LQ, DM), dtype=np.float32)
    attn = np.zeros((B, LQ, LK), dtype=np.float32)
    for core in range(NCORES):
        b, q0 = core // 4, QS * (core % 4)
        out[b, q0:q0 + QS] = res.results[core]["outp"]
        attn[b, q0:q0 + QS] = res.results[core]["attnT"].T
    if _trace:
        kernel.last_exec_time_ns = res.exec_time_ns
    return out, attn


# revision 11
# speedup vs baseline: 1.1875x; 1.1875x over previous
"""Distributed Trainium2 (Bass) kernel for additive (Bahdanau) attention.

Strategy
--------
The reference computes  scores[b,i,j] = sum_d v[d] * tanh(qp[b,i,d] + kp[b,j,d])
which is O(B*Lq*Lk*Dk) transcendental work (134M tanh) if done directly.
We replace tanh with a 5-term sine expansion

    tanh(s) ~= sum_m c_m sin(w_m s)            (fit on s in [-7.2, 7.2])

and use  sin(w(a+b)) = sin(wa+f)cos(wb-f) + cos(wa+f)sin(wb-f)  to turn the
score tensor into fp32r matmuls over 128 "trig feature" rows per frequency:

    scoresT = sum_m Qf_m^T @ Pf_m          [Lk, 256] per core

ACT's Sin is only valid on ~[-3.4, 3.4]; arguments are range-reduced on DVE
with a fused magic-number round (one tensor_scalar per pass).  For the lowest
frequency a phase shift f=pi/4 keeps all arguments inside [-0.5, 0.5] turns
(no reduction needed), which lets the PE start the score matmuls early.

Sharding: 8 cores = 2 batches x 4 query-blocks of 256 rows.  K/V (transposed
on host, bf16) are replicated per batch; no collectives.  attn is produced
transposed ([Lk, 256] bf16 per core) and transposed back on the host.
"""

import sys
import numpy as np

if "/opt/trn_rl_repo" not in sys.path:
    sys.path.insert(0, "/opt/trn_rl_repo")

import ml_dtypes

B, LQ, LK, DM, DK, DV = 2, 1024, 1024, 512, 64, 64
QS = 256          # query rows per core
NCORES = 8
M = 5             # sine terms
LN_EPS = 1e-6
MAGIC = 12582912.0  # 1.5 * 2^23: fused (x+MAGIC)-MAGIC == round(x) on DVE

# tanh(s) ~= sum c_m sin(om_m s), weighted LSQ fit on [-7.2, 7.2]
CS = np.array([1.20989, 0.273641, 0.08294, 0.024901, 0.006387])
OMS = np.array([0.365543, 1.109931, 1.887858, 2.738271, 3.786378])
OMP = OMS / (2 * np.pi)

_CACHE = {}

# F_all layout (free-axis offsets, elements): per-freq trig features
#   [Qf1 | Praw1 | Qf2 Qf3 Qf4 Qf5 | Praw2..5]
QO = [0, 1280, 2304, 3328, 4352]          # Q feature slice offsets (len LK)
PO = [1024, 5376, 5632, 5888, 6144]       # raw P sin slice offsets (len QS)
FTOT = 6400
UTOT = 5120   # round-path scratch: [uQ2..5 (4*LK) | uP2..5 (4*QS)]


def _build():
    """Build (and cache) the Bass graph."""
    if "nc" in _CACHE:
        return _CACHE["nc"]
    import concourse.bacc as bacc
    import concourse.mybir as mybir
    from concourse.tile import TileContext

    f32 = mybir.dt.float32
    f32r = mybir.dt.float32r
    bf16 = mybir.dt.bfloat16
    AF = mybir.ActivationFunctionType
    ALU = mybir.AluOpType

    nc = bacc.Bacc("TRN2", target_bir_lowering=False, debug=False,
                   num_devices=NCORES)

    qT_d = nc.dram_tensor("qT", [DM, QS], bf16, kind="ExternalInput").ap()
    qn_d = nc.dram_tensor("qn", [QS, DM], f32, kind="ExternalInput").ap()
    kT_d = nc.dram_tensor("kT", [DM, LK], bf16, kind="ExternalInput").ap()
    vT_d = nc.dram_tensor("vT", [DM, LK], bf16, kind="ExternalInput").ap()
    wq2_d = nc.dram_tensor("wq2", [DM, 128], bf16, kind="ExternalInput").ap()
    wk2_d = nc.dram_tensor("wk2", [DM, 128], bf16, kind="ExternalInput").ap()
    wv_d = nc.dram_tensor("wv", [DM, DV], bf16, kind="ExternalInput").ap()
    wfc_d = nc.dram_tensor("wfc", [DV, DM], f32, kind="ExternalInput").ap()
    pco_d = nc.dram_tensor("pco", [128, M], f32, kind="ExternalInput").ap()
    dvs_d = nc.dram_tensor("dvs", [128, 2], f32, kind="ExternalInput").ap()
    gam_d = nc.dram_tensor("gam", [1, DM], f32, kind="ExternalInput").ap()
    bet_d = nc.dram_tensor("bet", [1, DM], f32, kind="ExternalInput").ap()
    eye_d = nc.dram_tensor("eye", [64, 64], f32, kind="ExternalInput").ap()

    attn_d = nc.dram_tensor("attnT", [LK, QS], bf16, kind="ExternalOutput").ap()
    out_d = nc.dram_tensor("outp", [QS, DM], bf16, kind="ExternalOutput").ap()

    with TileContext(nc) as tc:
        import contextlib
        with contextlib.ExitStack() as ctx:
            const = ctx.enter_context(tc.tile_pool(name="const", bufs=1))
            work = ctx.enter_context(tc.tile_pool(name="work", bufs=1))
            small = ctx.enter_context(tc.tile_pool(name="small", bufs=2))
            ps1 = ctx.enter_context(tc.tile_pool(name="ps1", bufs=1, space="PSUM"))

            # ---------------- inputs -> SBUF ----------------
            wk2_sb = const.tile([128, 4, 128], bf16)
            nc.sync.dma_start(out=wk2_sb, in_=wk2_d.rearrange("(t p) m -> p t m", p=128))
            kT_sb = const.tile([128, 4, LK], bf16)
            nc.sync.dma_start(out=kT_sb, in_=kT_d.rearrange("(t p) l -> p t l", p=128))
            wq2_sb = const.tile([128, 4, 128], bf16)
            nc.sync.dma_start(out=wq2_sb, in_=wq2_d.rearrange("(t p) m -> p t m", p=128))
            qT_sb = const.tile([128, 4, QS], bf16)
            nc.sync.dma_start(out=qT_sb, in_=qT_d.rearrange("(t p) l -> p t l", p=128))
            wv_sb = const.tile([128, 4, DV], bf16)
            nc.sync.dma_start(out=wv_sb, in_=wv_d.rearrange("(t p) m -> p t m", p=128))
            vT_sb = const.tile([128, 4, LK], bf16)
            nc.sync.dma_start(out=vT_sb, in_=vT_d.rearrange("(t p) l -> p t l", p=128))
            pco_sb = const.tile([128, M], f32)
            nc.sync.dma_start(out=pco_sb, in_=pco_d)
            dvs_sb = const.tile([128, 2], f32)   # col 0: f1 delta rows
            nc.sync.dma_start(out=dvs_sb, in_=dvs_d)
            eye_sb = const.tile([64, 64], f32)
            nc.sync.dma_start(out=eye_sb, in_=eye_d)
            qn_sb = const.tile([128, 2, DM], f32)
            nc.sync.dma_start(out=qn_sb, in_=qn_d.rearrange("(t p) l -> p t l", p=128))
            wfc_f = const.tile([64, DM], f32)
            nc.sync.dma_start(out=wfc_f, in_=wfc_d)
            gam_sb = const.tile([128, DM], f32)
            gam_b = type(gam_d)(tensor=gam_d.tensor, offset=gam_d.offset,
                                ap=[[0, 128]] + gam_d.ap[1:])
            nc.sync.dma_start(out=gam_sb, in_=gam_b)
            bet_sb = const.tile([128, DM], f32)
            bet_b = type(bet_d)(tensor=bet_d.tensor, offset=bet_d.offset,
                                ap=[[0, 128]] + bet_d.ap[1:])
            nc.sync.dma_start(out=bet_sb, in_=bet_b)

            wfc_r = const.tile([64, DM], f32r)
            nc.vector.tensor_copy(wfc_r[:], wfc_f[:])
            dvQ = const.tile([128, 1], f32)   # f2..5 Q: cos-rows 0.25, sin 0
            nc.vector.memset(dvQ[0:64, :], 0.25)
            nc.vector.memset(dvQ[64:128, :], 0.0)
            dvP = const.tile([128, 1], f32)   # f2..5 P: sin-rows 0, cos 0.25
            nc.vector.memset(dvP[0:64, :], 0.0)
            nc.vector.memset(dvP[64:128, :], 0.25)
            ones_f = const.tile([128, 1], f32)
            nc.vector.memset(ones_f, 1.0)
            ones_r = const.tile([128, 1], f32r)
            nc.vector.tensor_copy(ones_r[:], ones_f[:])
            ones1_f = const.tile([1, 128], f32)
            nc.vector.memset(ones1_f, 1.0)
            ones1_r = const.tile([1, 128], f32r)
            nc.vector.tensor_copy(ones1_r[:], ones1_f[:])
            eps_sb = const.tile([128, 1], f32)
            nc.vector.memset(eps_sb, LN_EPS)

            # ---------------- projections (bf16 matmuls) ----------------
            kp2_ps = ps1.tile([128, 2048], f32, tag="kq")   # 4 banks (reused)
            for t in range(4):
                for h in range(2):
                    nc.tensor.matmul(kp2_ps[:, h * 512:(h + 1) * 512],
                                     wk2_sb[:, t, :],
                                     kT_sb[:, t, h * 512:(h + 1) * 512],
                                     start=(t == 0), stop=(t == 3))
            qp2_ps = ps1.tile([128, QS], f32, tag="qtb")
            for t in range(4):
                nc.tensor.matmul(qp2_ps[:], wq2_sb[:, t, :], qT_sb[:, t, :],
                                 start=(t == 0), stop=(t == 3))
            vp_ps = ps1.tile([64, LK], f32, tag="vpo")
            for t in range(4):
                for h in range(2):
                    nc.tensor.matmul(vp_ps[:, h * 512:(h + 1) * 512],
                                     wv_sb[:, t, :],
                                     vT_sb[:, t, h * 512:(h + 1) * 512],
                                     start=(t == 0), stop=(t == 3))

            # ---------------- trig features -----------------------------
            rarg = work.tile([128, FTOT], f32)    # pre-sin arguments
            fall = work.tile([128, FTOT], f32r)   # sin outputs (features)
            pfin = work.tile([128, M, QS], f32r)  # coef-scaled P features

            # freq 1: no range reduction (phase trick keeps |u| < 0.5)
            nc.vector.tensor_scalar(out=rarg[:, PO[0]:PO[0] + QS], in0=qp2_ps[:],
                                    scalar1=float(OMP[0]), scalar2=dvs_sb[:, 0:1],
                                    op0=ALU.mult, op1=ALU.add)
            nc.vector.tensor_scalar(out=rarg[:, QO[0]:QO[0] + LK],
                                    in0=kp2_ps[:, 0:LK],
                                    scalar1=float(OMP[0]), scalar2=dvs_sb[:, 0:1],
                                    op0=ALU.mult, op1=ALU.add)
            nc.scalar.activation(out=fall[:, PO[0]:PO[0] + QS],
                                 in_=rarg[:, PO[0]:PO[0] + QS],
                                 func=AF.Sin, scale=float(2 * np.pi))
            nc.scalar.activation(out=fall[:, QO[0]:QO[0] + LK],
                                 in_=rarg[:, QO[0]:QO[0] + LK],
                                 func=AF.Sin, scale=float(2 * np.pi))
            nc.vector.tensor_scalar(out=pfin[:, 0, :],
                                    in0=fall[:, PO[0]:PO[0] + QS],
                                    scalar1=pco_sb[:, 0:1], scalar2=None,
                                    op0=ALU.mult)

            # freqs 2..5: fused-round range reduction
            uu = work.tile([128, UTOT], f32)
            nn = work.tile([128, UTOT], f32)
            for i, m in enumerate(range(1, M)):
                nc.vector.tensor_scalar(out=uu[:, i * LK:(i + 1) * LK],
                                        in0=kp2_ps[:, 0:LK],
                                        scalar1=float(OMP[m]), scalar2=dvQ[:],
                                        op0=ALU.mult, op1=ALU.add)
            for i, m in enumerate(range(1, M)):
                nc.vector.tensor_scalar(
                    out=uu[:, 4 * LK + i * QS:4 * LK + (i + 1) * QS],
                    in0=qp2_ps[:],
                    scalar1=float(OMP[m]), scalar2=dvP[:],
                    op0=ALU.mult, op1=ALU.add)
            nc.vector.tensor_scalar(out=nn[:], in0=uu[:], scalar1=MAGIC,
                                    scalar2=-MAGIC, op0=ALU.add, op1=ALU.add)
            nc.vector.tensor_tensor(out=rarg[:, 1280:6400], in0=uu[:], in1=nn[:],
                                    op=ALU.subtract)
            for m in range(1, M):
                nc.scalar.activation(out=fall[:, QO[m]:QO[m] + LK],
                                     in_=rarg[:, QO[m]:QO[m] + LK],
                                     func=AF.Sin, scale=float(2 * np.pi))
            nc.scalar.activation(out=fall[:, PO[1]:PO[1] + 4 * QS],
                                 in_=rarg[:, PO[1]:PO[1] + 4 * QS],
                                 func=AF.Sin, scale=float(2 * np.pi))
            for m in range(1, M):
                nc.vector.tensor_scalar(out=pfin[:, m, :],
                                        in0=fall[:, PO[m]:PO[m] + QS],
                                        scalar1=pco_sb[:, m:m + 1], scalar2=None,
                                        op0=ALU.mult)

            # -------- vp transpose: vpT [64, LK] -> vp [LK-tiles, 64] ----
            vpT_sb = work.tile([64, LK], f32)
            nc.vector.tensor_copy(vpT_sb[:], vp_ps[:])
            vp_sb = work.tile([128, 8, 64], f32r)
            tr_ps = ps1.tile([128, 8, 64], f32, tag="qtb")
            for kc in range(8):
                nc.tensor.transpose(tr_ps[:, kc, :],
                                    vpT_sb[:, kc * 128:(kc + 1) * 128],
                                    eye_sb[:])
            nc.vector.tensor_copy(vp_sb[:], tr_ps[:])

            # ---------------- scores ------------------------------------
            sc_ps = ps1.tile([128, 2048], f32, tag="kq")   # 8 chunks of 256
            for kc in range(8):
                for m in range(M):
                    nc.tensor.matmul(
                        sc_ps[:, kc * QS:(kc + 1) * QS],
                        fall[:, QO[m] + kc * 128:QO[m] + (kc + 1) * 128],
                        pfin[:, m, :],
                        start=(m == 0), stop=(m == M - 1))

            # ---------------- softmax / av -------------------------------
            du_ps = ps1.tile([64, QS], f32, tag="du")   # attn @ vp accumulator
            den_ps = ps1.tile([1, QS], f32, tag="qtb")  # softmax denominators
            expT = work.tile([128, 8, QS], f32r)
            for kp_ in range(4):  # exp over kc pairs
                nc.scalar.activation(out=expT[:, 2 * kp_:2 * kp_ + 2, :],
                                     in_=sc_ps[:, 2 * kp_ * QS:(2 * kp_ + 2) * QS],
                                     func=AF.Exp, scale=1.0)
            for kc in range(8):
                nc.tensor.matmul(den_ps[:], ones_r[:], expT[:, kc, :],
                                 start=(kc == 0), stop=(kc == 7))
            for kc in range(8):
                nc.tensor.matmul(du_ps[:], vp_sb[:, kc, :], expT[:, kc, :],
                                 start=(kc == 0), stop=(kc == 7))

            rec_f = small.tile([1, QS], f32)
            nc.vector.reciprocal(out=rec_f[:], in_=den_ps[:])
            rec_r = small.tile([1, QS], f32r)
            nc.vector.tensor_copy(rec_r[:], rec_f[:])
            bc_ps = ps1.tile([128, QS], f32, tag="qtb")
            nc.tensor.matmul(bc_ps[:], ones1_r[:], rec_r[:], start=True, stop=True)
            bc_sb = work.tile([128, QS], f32)
            nc.vector.tensor_copy(bc_sb[:], bc_ps[:])

            # attn output (normalized, bf16, transposed)
            for kc in range(8):
                at_sb = small.tile([128, QS], bf16, tag="at")
                nc.gpsimd.tensor_tensor(out=at_sb[:],
                                        in0=expT[:, kc, :].bitcast(f32),
                                        in1=bc_sb[:], op=ALU.mult)
                nc.sync.dma_start(out=attn_d[kc * 128:(kc + 1) * 128, :],
                                  in_=at_sb[:])

            avn = work.tile([64, QS], f32r)
            nc.vector.tensor_tensor(out=avn[:], in0=du_ps[:],
                                    in1=bc_sb[0:64, :], op=ALU.mult)

            # ---------------- fc + residual + layernorm -----------------
            for qc in range(2):
                o_ps = ps1.tile([128, DM], f32, tag="vpo")
                nc.tensor.matmul(o_ps[:], avn[:, qc * 128:(qc + 1) * 128],
                                 wfc_r[:], start=True, stop=True)
                o1 = small.tile([128, DM], f32, tag="o1")
                nc.vector.tensor_tensor(out=o1[:], in0=o_ps[:],
                                        in1=qn_sb[:, qc, :], op=ALU.add)
                st = small.tile([128, 6], f32, tag="st")
                nc.vector.bn_stats(out=st[:], in_=o1[:])
                mv = small.tile([128, 2], f32, tag="mv")
                nc.vector.bn_aggr(out=mv[:], in_=st[:])
                sq = small.tile([128, 1], f32, tag="sq")
                nc.scalar.activation(out=sq[:], in_=mv[:, 1:2], func=AF.Sqrt,
                                     scale=1.0, bias=eps_sb[:])
                rs = small.tile([128, 1], f32, tag="rs")
                nc.vector.reciprocal(out=rs[:], in_=sq[:])
                o2 = small.tile([128, DM], f32, tag="o2")
                nc.vector.tensor_scalar(out=o2[:], in0=o1[:],
                                        scalar1=mv[:, 0:1], scalar2=rs[:],
                                        op0=ALU.subtract, op1=ALU.mult)
                o3 = small.tile([128, DM], f32, tag="o3")
                nc.gpsimd.tensor_tensor(out=o3[:], in0=o2[:], in1=gam_sb[:],
                                        op=ALU.mult)
                o4 = small.tile([128, DM], bf16, tag="o4")
                nc.gpsimd.tensor_tensor(out=o4[:], in0=o3[:], in1=bet_sb[:],
                                        op=ALU.add)
                nc.sync.dma_start(
                    out=out_d.rearrange("(t p) l -> p t l", p=128)[:, qc, :],
                    in_=o4[:])

    nc.finalize()
    _CACHE["nc"] = nc
    return nc


def _prep_inputs(q, k, v, Wq, Wk, Wv, v_param, Wfc, ln_gamma, ln_beta):
    bf = ml_dtypes.bfloat16
    wq2 = np.ascontiguousarray(np.concatenate([Wq, Wq], axis=1)).astype(bf)
    wk2 = np.ascontiguousarray(np.concatenate([Wk, Wk], axis=1)).astype(bf)
    wv = np.ascontiguousarray(Wv).astype(bf)
    wfc = np.ascontiguousarray(Wfc).astype(np.float32)
    pco = np.zeros((128, M), dtype=np.float32)
    for m in range(M):
        s = -1.0 if m == 0 else 1.0   # f1 sin-rows pair with Q's "-cos" rows
        pco[0:64, m] = (s * CS[m] * v_param).astype(np.float32)
        pco[64:128, m] = (CS[m] * v_param).astype(np.float32)
    dvs = np.zeros((128, 2), dtype=np.float32)
    dvs[0:64, 0] = -0.125   # f1: Q "-cos" rows / P sin rows
    dvs[64:128, 0] = 0.125  # f1: Q sin rows / P cos rows
    gam = np.ascontiguousarray(ln_gamma.reshape(1, DM)).astype(np.float32)
    bet = np.ascontiguousarray(ln_beta.reshape(1, DM)).astype(np.float32)
    eye = np.eye(64, dtype=np.float32)

    kT = [np.ascontiguousarray(k[b].T).astype(bf) for b in range(B)]
    vT = [np.ascontiguousarray(v[b].T).astype(bf) for b in range(B)]

    in_maps = []
    for core in range(NCORES):
        b, q0 = core // 4, QS * (core % 4)
        qs = q[b, q0:q0 + QS]
        in_maps.append({
            "qT": np.ascontiguousarray(qs.T).astype(bf),
            "qn": np.ascontiguousarray(qs).astype(np.float32),
            "kT": kT[b],
            "vT": vT[b],
            "wq2": wq2, "wk2": wk2, "wv": wv, "wfc": wfc,
            "pco": pco, "dvs": dvs, "gam": gam, "bet": bet, "eye": eye,
        })
    return in_maps


def kernel(q, k, v, Wq, Wk, Wv, v_param, Wfc, ln_gamma, ln_beta,
           _trace=False):
    q = np.asarray(q); k = np.asarray(k); v = np.asarray(v)
    Wq = np.asarray(Wq); Wk = np.asarray(Wk); Wv = np.asarray(Wv)
    v_param = np.asarray(v_param); Wfc = np.asarray(Wfc)
    ln_gamma = np.asarray(ln_gamma); ln_beta = np.asarray(ln_beta)

    from concourse.bass_utils import run_bass_kernel_spmd

    nc = _build()
    in_maps = _prep_inputs(q, k, v, Wq, Wk, Wv, v_param, Wfc, ln_gamma, ln_beta)
    res = run_bass_kernel_spmd(nc, in_maps, core_ids=list(range(NCORES)),
                               trace=_trace)

    out = np.zeros((B, LQ, DM), dtype=np.float32)
    attn = np.zeros((B, LQ, LK), dtype=np.float32)
    for core in range(NCORES):
        b, q0 = core // 4, QS * (core % 4)
        out[b, q0:q0 + QS] = res.results[core]["outp"].astype(np.float32)
        attn[b, q0:q0 + QS] = res.results[core]["attnT"].astype(np.float32).T
    if _trace:
        kernel.last_exec_time_ns = res.exec_time_ns
    return out, attn


# revision 12
# speedup vs baseline: 1.1928x; 1.0045x over previous
"""Distributed Trainium2 (Bass) kernel for additive (Bahdanau) attention.

Strategy
--------
The reference computes  scores[b,i,j] = sum_d v[d] * tanh(qp[b,i,d] + kp[b,j,d])
which is O(B*Lq*Lk*Dk) transcendental work (134M tanh) if done directly.
We replace tanh with a 5-term sine expansion

    tanh(s) ~= sum_m c_m sin(w_m s)            (fit on s in [-7.2, 7.2])

and use  sin(w(a+b)) = sin(wa+f)cos(wb-f) + cos(wa+f)sin(wb-f)  to turn the
score tensor into fp32r matmuls over 128 "trig feature" rows per frequency:

    scoresT = sum_m Qf_m^T @ Pf_m          [Lk, 256] per core

ACT's Sin is only valid on ~[-3.4, 3.4]; arguments are range-reduced on DVE
with a fused magic-number round (one tensor_scalar per pass).  For the lowest
frequency a phase shift f=pi/4 keeps all arguments inside [-0.5, 0.5] turns
(no reduction needed), which lets the PE start the score matmuls early.

Sharding: 8 cores = 2 batches x 4 query-blocks of 256 rows.  K/V (transposed
on host, bf16) are replicated per batch; no collectives.  attn is produced
transposed ([Lk, 256] bf16 per core) and transposed back on the host.
"""

import sys
import numpy as np

if "/opt/trn_rl_repo" not in sys.path:
    sys.path.insert(0, "/opt/trn_rl_repo")

import ml_dtypes

B, LQ, LK, DM, DK, DV = 2, 1024, 1024, 512, 64, 64
QS = 256          # query rows per core
NCORES = 8
M = 5             # sine terms
LN_EPS = 1e-6
MAGIC = 12582912.0  # 1.5 * 2^23: fused (x+MAGIC)-MAGIC == round(x) on DVE

# tanh(s) ~= sum c_m sin(om_m s), weighted LSQ fit on [-7.2, 7.2]
CS = np.array([1.20989, 0.273641, 0.08294, 0.024901, 0.006387])
OMS = np.array([0.365543, 1.109931, 1.887858, 2.738271, 3.786378])
OMP = OMS / (2 * np.pi)

_CACHE = {}

# F_all layout (free-axis offsets, elements): per-freq trig features
#   [Qf1 | Praw1 | Qf2 Qf3 Qf4 Qf5 | Praw2..5]
QO = [0, 1280, 2304, 3328, 4352]          # Q feature slice offsets (len LK)
PO = [1024, 5376, 5632, 5888, 6144]       # raw P sin slice offsets (len QS)
FTOT = 6400
UTOT = 5120   # round-path scratch: [uQ2..5 (4*LK) | uP2..5 (4*QS)]


def _build():
    """Build (and cache) the Bass graph."""
    if "nc" in _CACHE:
        return _CACHE["nc"]
    import concourse.bacc as bacc
    import concourse.mybir as mybir
    from concourse.tile import TileContext

    f32 = mybir.dt.float32
    f32r = mybir.dt.float32r
    bf16 = mybir.dt.bfloat16
    AF = mybir.ActivationFunctionType
    ALU = mybir.AluOpType

    nc = bacc.Bacc("TRN2", target_bir_lowering=False, debug=False,
                   num_devices=NCORES)

    qT_d = nc.dram_tensor("qT", [DM, QS], bf16, kind="ExternalInput").ap()
    qn_d = nc.dram_tensor("qn", [QS, DM], f32, kind="ExternalInput").ap()
    kT_d = nc.dram_tensor("kT", [DM, LK], bf16, kind="ExternalInput").ap()
    vT_d = nc.dram_tensor("vT", [DM, LK], bf16, kind="ExternalInput").ap()
    wq2_d = nc.dram_tensor("wq2", [DM, 128], bf16, kind="ExternalInput").ap()
    wk2_d = nc.dram_tensor("wk2", [DM, 128], bf16, kind="ExternalInput").ap()
    wv_d = nc.dram_tensor("wv", [DM, DV], bf16, kind="ExternalInput").ap()
    wfc_d = nc.dram_tensor("wfc", [DV, DM], f32, kind="ExternalInput").ap()
    pco_d = nc.dram_tensor("pco", [128, M], f32, kind="ExternalInput").ap()
    dvs_d = nc.dram_tensor("dvs", [128, 2], f32, kind="ExternalInput").ap()
    gam_d = nc.dram_tensor("gam", [1, DM], f32, kind="ExternalInput").ap()
    bet_d = nc.dram_tensor("bet", [1, DM], f32, kind="ExternalInput").ap()
    eye_d = nc.dram_tensor("eye", [64, 64], f32, kind="ExternalInput").ap()

    attn_d = nc.dram_tensor("attnT", [LK, QS], bf16, kind="ExternalOutput").ap()
    out_d = nc.dram_tensor("outp", [QS, DM], bf16, kind="ExternalOutput").ap()

    with TileContext(nc) as tc:
        import contextlib
        with contextlib.ExitStack() as ctx:
            const = ctx.enter_context(tc.tile_pool(name="const", bufs=1))
            work = ctx.enter_context(tc.tile_pool(name="work", bufs=1))
            small = ctx.enter_context(tc.tile_pool(name="small", bufs=2))
            ps1 = ctx.enter_context(tc.tile_pool(name="ps1", bufs=1, space="PSUM"))

            # ---------------- inputs -> SBUF (small/critical first) ------
            pco_sb = const.tile([128, M], f32)
            nc.sync.dma_start(out=pco_sb, in_=pco_d)
            dvs_sb = const.tile([128, 2], f32)   # col 0: f1 delta rows
            nc.sync.dma_start(out=dvs_sb, in_=dvs_d)
            eye_sb = const.tile([64, 64], f32)
            nc.sync.dma_start(out=eye_sb, in_=eye_d)
            wk2_sb = const.tile([128, 4, 128], bf16)
            nc.sync.dma_start(out=wk2_sb, in_=wk2_d.rearrange("(t p) m -> p t m", p=128))
            wq2_sb = const.tile([128, 4, 128], bf16)
            nc.sync.dma_start(out=wq2_sb, in_=wq2_d.rearrange("(t p) m -> p t m", p=128))
            wv_sb = const.tile([128, 4, DV], bf16)
            nc.sync.dma_start(out=wv_sb, in_=wv_d.rearrange("(t p) m -> p t m", p=128))
            wfc_f = const.tile([64, DM], f32)
            nc.sync.dma_start(out=wfc_f, in_=wfc_d)
            qT_sb = const.tile([128, 4, QS], bf16)
            nc.sync.dma_start(out=qT_sb, in_=qT_d.rearrange("(t p) l -> p t l", p=128))
            kT_sb = const.tile([128, 4, LK], bf16)
            nc.sync.dma_start(out=kT_sb, in_=kT_d.rearrange("(t p) l -> p t l", p=128))
            vT_sb = const.tile([128, 4, LK], bf16)
            nc.sync.dma_start(out=vT_sb, in_=vT_d.rearrange("(t p) l -> p t l", p=128))
            qn_sb = const.tile([128, 2, DM], f32)
            nc.sync.dma_start(out=qn_sb, in_=qn_d.rearrange("(t p) l -> p t l", p=128))
            gam_sb = const.tile([128, DM], f32)
            gam_b = type(gam_d)(tensor=gam_d.tensor, offset=gam_d.offset,
                                ap=[[0, 128]] + gam_d.ap[1:])
            nc.sync.dma_start(out=gam_sb, in_=gam_b)
            bet_sb = const.tile([128, DM], f32)
            bet_b = type(bet_d)(tensor=bet_d.tensor, offset=bet_d.offset,
                                ap=[[0, 128]] + bet_d.ap[1:])
            nc.sync.dma_start(out=bet_sb, in_=bet_b)

            wfc_r = const.tile([64, DM], f32r)
            nc.vector.tensor_copy(wfc_r[:], wfc_f[:])
            dvQ = const.tile([128, 1], f32)   # f2..5 Q: cos-rows 0.25, sin 0
            nc.vector.memset(dvQ[0:64, :], 0.25)
            nc.vector.memset(dvQ[64:128, :], 0.0)
            dvP = const.tile([128, 1], f32)   # f2..5 P: sin-rows 0, cos 0.25
            nc.vector.memset(dvP[0:64, :], 0.0)
            nc.vector.memset(dvP[64:128, :], 0.25)
            ones_f = const.tile([128, 1], f32)
            nc.vector.memset(ones_f, 1.0)
            ones_r = const.tile([128, 1], f32r)
            nc.vector.tensor_copy(ones_r[:], ones_f[:])
            ones1_f = const.tile([1, 128], f32)
            nc.vector.memset(ones1_f, 1.0)
            ones1_r = const.tile([1, 128], f32r)
            nc.vector.tensor_copy(ones1_r[:], ones1_f[:])
            eps_sb = const.tile([128, 1], f32)
            nc.vector.memset(eps_sb, LN_EPS)

            # ---------------- projections (bf16 matmuls) ----------------
            kp2_ps = ps1.tile([128, 2048], f32, tag="kq")   # 4 banks (reused)
            for t in range(4):
                for h in range(2):
                    nc.tensor.matmul(kp2_ps[:, h * 512:(h + 1) * 512],
                                     wk2_sb[:, t, :],
                                     kT_sb[:, t, h * 512:(h + 1) * 512],
                                     start=(t == 0), stop=(t == 3))
            qp2_ps = ps1.tile([128, QS], f32, tag="qtb")
            for t in range(4):
                nc.tensor.matmul(qp2_ps[:], wq2_sb[:, t, :], qT_sb[:, t, :],
                                 start=(t == 0), stop=(t == 3))
            vp_ps = ps1.tile([64, LK], f32, tag="vpo")
            for t in range(4):
                for h in range(2):
                    nc.tensor.matmul(vp_ps[:, h * 512:(h + 1) * 512],
                                     wv_sb[:, t, :],
                                     vT_sb[:, t, h * 512:(h + 1) * 512],
                                     start=(t == 0), stop=(t == 3))

            # ---------------- trig features -----------------------------
            kp2_sb = work.tile([128, LK], f32)
            nc.vector.tensor_copy(kp2_sb[:], kp2_ps[:, 0:LK])
            qp2_sb = work.tile([128, QS], f32)
            nc.vector.tensor_copy(qp2_sb[:], qp2_ps[:])
            rarg = work.tile([128, FTOT], f32)    # pre-sin arguments
            fall = work.tile([128, FTOT], f32r)   # sin outputs (features)
            pfin = work.tile([128, M, QS], f32r)  # coef-scaled P features

            # freq 1: no range reduction (phase trick keeps |u| < 0.5)
            nc.vector.tensor_scalar(out=rarg[:, PO[0]:PO[0] + QS], in0=qp2_sb[:],
                                    scalar1=float(OMP[0]), scalar2=dvs_sb[:, 0:1],
                                    op0=ALU.mult, op1=ALU.add)
            nc.vector.tensor_scalar(out=rarg[:, QO[0]:QO[0] + LK],
                                    in0=kp2_sb[:],
                                    scalar1=float(OMP[0]), scalar2=dvs_sb[:, 0:1],
                                    op0=ALU.mult, op1=ALU.add)
            nc.scalar.activation(out=fall[:, PO[0]:PO[0] + QS],
                                 in_=rarg[:, PO[0]:PO[0] + QS],
                                 func=AF.Sin, scale=float(2 * np.pi))
            nc.scalar.activation(out=fall[:, QO[0]:QO[0] + LK],
                                 in_=rarg[:, QO[0]:QO[0] + LK],
                                 func=AF.Sin, scale=float(2 * np.pi))
            nc.vector.tensor_scalar(out=pfin[:, 0, :],
                                    in0=fall[:, PO[0]:PO[0] + QS],
                                    scalar1=pco_sb[:, 0:1], scalar2=None,
                                    op0=ALU.mult)

            # freqs 2..5: fused-round range reduction
            uu = work.tile([128, UTOT], f32)
            nn = work.tile([128, UTOT], f32)
            for i, m in enumerate(range(1, M)):
                nc.vector.tensor_scalar(out=uu[:, i * LK:(i + 1) * LK],
                                        in0=kp2_sb[:],
                                        scalar1=float(OMP[m]), scalar2=dvQ[:],
                                        op0=ALU.mult, op1=ALU.add)
            for i, m in enumerate(range(1, M)):
                nc.vector.tensor_scalar(
                    out=uu[:, 4 * LK + i * QS:4 * LK + (i + 1) * QS],
                    in0=qp2_sb[:],
                    scalar1=float(OMP[m]), scalar2=dvP[:],
                    op0=ALU.mult, op1=ALU.add)
            nc.vector.tensor_scalar(out=nn[:], in0=uu[:], scalar1=MAGIC,
                                    scalar2=-MAGIC, op0=ALU.add, op1=ALU.add)
            nc.vector.tensor_tensor(out=rarg[:, 1280:4864], in0=uu[:, 0:3584],
                                    in1=nn[:, 0:3584], op=ALU.subtract)
            nc.gpsimd.tensor_tensor(out=rarg[:, 4864:6400], in0=uu[:, 3584:5120],
                                    in1=nn[:, 3584:5120], op=ALU.subtract)
            for m in range(1, M):
                nc.scalar.activation(out=fall[:, QO[m]:QO[m] + LK],
                                     in_=rarg[:, QO[m]:QO[m] + LK],
                                     func=AF.Sin, scale=float(2 * np.pi))
            nc.scalar.activation(out=fall[:, PO[1]:PO[1] + 4 * QS],
                                 in_=rarg[:, PO[1]:PO[1] + 4 * QS],
                                 func=AF.Sin, scale=float(2 * np.pi))
            for m in range(1, M):
                nc.vector.tensor_scalar(out=pfin[:, m, :],
                                        in0=fall[:, PO[m]:PO[m] + QS],
                                        scalar1=pco_sb[:, m:m + 1], scalar2=None,
                                        op0=ALU.mult)

            # -------- vp transpose: vpT [64, LK] -> vp [LK-tiles, 64] ----
            vpT_sb = work.tile([64, LK], f32)
            nc.vector.tensor_copy(vpT_sb[:], vp_ps[:])
            vp_sb = work.tile([128, 8, 64], f32r)
            tr_ps = ps1.tile([128, 8, 64], f32, tag="qtb")
            for kc in range(8):
                nc.tensor.transpose(tr_ps[:, kc, :],
                                    vpT_sb[:, kc * 128:(kc + 1) * 128],
                                    eye_sb[:])
            nc.vector.tensor_copy(vp_sb[:], tr_ps[:])

            # ---------------- scores ------------------------------------
            sc_ps = ps1.tile([128, 2048], f32, tag="kq")   # 8 chunks of 256
            for kc in range(8):
                for m in range(M):
                    nc.tensor.matmul(
                        sc_ps[:, kc * QS:(kc + 1) * QS],
                        fall[:, QO[m] + kc * 128:QO[m] + (kc + 1) * 128],
                        pfin[:, m, :],
                        start=(m == 0), stop=(m == M - 1))

            # ---------------- softmax / av -------------------------------
            du_ps = ps1.tile([64, QS], f32, tag="du")   # attn @ vp accumulator
            den_ps = ps1.tile([1, QS], f32, tag="qtb")  # softmax denominators
            expT = work.tile([128, 8, QS], f32r)
            for kp_ in range(4):  # exp over kc pairs
                nc.scalar.activation(out=expT[:, 2 * kp_:2 * kp_ + 2, :],
                                     in_=sc_ps[:, 2 * kp_ * QS:(2 * kp_ + 2) * QS],
                                     func=AF.Exp, scale=1.0)
            for kc in range(8):
                nc.tensor.matmul(den_ps[:], ones_r[:], expT[:, kc, :],
                                 start=(kc == 0), stop=(kc == 7))
            for kc in range(8):
                nc.tensor.matmul(du_ps[:], vp_sb[:, kc, :], expT[:, kc, :],
                                 start=(kc == 0), stop=(kc == 7))

            rec_f = small.tile([1, QS], f32)
            nc.vector.reciprocal(out=rec_f[:], in_=den_ps[:])
            rec_r = small.tile([1, QS], f32r)
            nc.vector.tensor_copy(rec_r[:], rec_f[:])
            bc_ps = ps1.tile([128, QS], f32, tag="qtb")
            nc.tensor.matmul(bc_ps[:], ones1_r[:], rec_r[:], start=True, stop=True)
            bc_sb = work.tile([128, QS], f32)
            nc.vector.tensor_copy(bc_sb[:], bc_ps[:])

            # attn output (normalized, bf16, transposed)
            for kc in range(8):
                at_sb = small.tile([128, QS], bf16, tag="at")
                eng = nc.gpsimd if kc % 2 == 0 else nc.vector
                eng.tensor_tensor(out=at_sb[:],
                                  in0=expT[:, kc, :].bitcast(f32),
                                  in1=bc_sb[:], op=ALU.mult)
                nc.sync.dma_start(out=attn_d[kc * 128:(kc + 1) * 128, :],
                                  in_=at_sb[:])

            avn = work.tile([64, QS], f32r)
            nc.vector.tensor_tensor(out=avn[:], in0=du_ps[:],
                                    in1=bc_sb[0:64, :], op=ALU.mult)

            # ---------------- fc + residual + layernorm -----------------
            for qc in range(2):
                o_ps = ps1.tile([128, DM], f32, tag="vpo")
                nc.tensor.matmul(o_ps[:], avn[:, qc * 128:(qc + 1) * 128],
                                 wfc_r[:], start=True, stop=True)
                o1 = small.tile([128, DM], f32, tag="o1")
                nc.vector.tensor_tensor(out=o1[:], in0=o_ps[:],
                                        in1=qn_sb[:, qc, :], op=ALU.add)
                st = small.tile([128, 6], f32, tag="st")
                nc.vector.bn_stats(out=st[:], in_=o1[:])
                mv = small.tile([128, 2], f32, tag="mv")
                nc.vector.bn_aggr(out=mv[:], in_=st[:])
                sq = small.tile([128, 1], f32, tag="sq")
                nc.scalar.activation(out=sq[:], in_=mv[:, 1:2], func=AF.Sqrt,
                                     scale=1.0, bias=eps_sb[:])
                rs = small.tile([128, 1], f32, tag="rs")
                nc.vector.reciprocal(out=rs[:], in_=sq[:])
                o2 = small.tile([128, DM], f32, tag="o2")
                nc.vector.tensor_scalar(out=o2[:], in0=o1[:],
                                        scalar1=mv[:, 0:1], scalar2=rs[:],
                                        op0=ALU.subtract, op1=ALU.mult)
                o3 = small.tile([128, DM], f32, tag="o3")
                nc.vector.tensor_tensor(out=o3[:], in0=o2[:], in1=gam_sb[:],
                                        op=ALU.mult)
                o4 = small.tile([128, DM], bf16, tag="o4")
                nc.vector.tensor_tensor(out=o4[:], in0=o3[:], in1=bet_sb[:],
                                        op=ALU.add)
                nc.sync.dma_start(
                    out=out_d.rearrange("(t p) l -> p t l", p=128)[:, qc, :],
                    in_=o4[:])

    nc.finalize()
    _CACHE["nc"] = nc
    return nc


def _prep_inputs(q, k, v, Wq, Wk, Wv, v_param, Wfc, ln_gamma, ln_beta):
    bf = ml_dtypes.bfloat16
    wq2 = np.ascontiguousarray(np.concatenate([Wq, Wq], axis=1)).astype(bf)
    wk2 = np.ascontiguousarray(np.concatenate([Wk, Wk], axis=1)).astype(bf)
    wv = np.ascontiguousarray(Wv).astype(bf)
    wfc = np.ascontiguousarray(Wfc).astype(np.float32)
    pco = np.zeros((128, M), dtype=np.float32)
    for m in range(M):
        s = -1.0 if m == 0 else 1.0   # f1 sin-rows pair with Q's "-cos" rows
        pco[0:64, m] = (s * CS[m] * v_param).astype(np.float32)
        pco[64:128, m] = (CS[m] * v_param).astype(np.float32)
    dvs = np.zeros((128, 2), dtype=np.float32)
    dvs[0:64, 0] = -0.125   # f1: Q "-cos" rows / P sin rows
    dvs[64:128, 0] = 0.125  # f1: Q sin rows / P cos rows
    gam = np.ascontiguousarray(ln_gamma.reshape(1, DM)).astype(np.float32)
    bet = np.ascontiguousarray(ln_beta.reshape(1, DM)).astype(np.float32)
    eye = np.eye(64, dtype=np.float32)

    kT = [np.ascontiguousarray(k[b].T).astype(bf) for b in range(B)]
    vT = [np.ascontiguousarray(v[b].T).astype(bf) for b in range(B)]

    in_maps = []
    for core in range(NCORES):
        b, q0 = core // 4, QS * (core % 4)
        qs = q[b, q0:q0 + QS]
        in_maps.append({
            "qT": np.ascontiguousarray(qs.T).astype(bf),
            "qn": np.ascontiguousarray(qs).astype(np.float32),
            "kT": kT[b],
            "vT": vT[b],
            "wq2": wq2, "wk2": wk2, "wv": wv, "wfc": wfc,
            "pco": pco, "dvs": dvs, "gam": gam, "bet": bet, "eye": eye,
        })
    return in_maps


def kernel(q, k, v, Wq, Wk, Wv, v_param, Wfc, ln_gamma, ln_beta,
           _trace=False):
    q = np.asarray(q); k = np.asarray(k); v = np.asarray(v)
    Wq = np.asarray(Wq); Wk = np.asarray(Wk); Wv = np.asarray(Wv)
    v_param = np.asarray(v_param); Wfc = np.asarray(Wfc)
    ln_gamma = np.asarray(ln_gamma); ln_beta = np.asarray(ln_beta)

    from concourse.bass_utils import run_bass_kernel_spmd

    nc = _build()
    in_maps = _prep_inputs(q, k, v, Wq, Wk, Wv, v_param, Wfc, ln_gamma, ln_beta)
    res = run_bass_kernel_spmd(nc, in_maps, core_ids=list(range(NCORES)),
                               trace=_trace)

    out = np.zeros((B, LQ, DM), dtype=np.float32)
    attn = np.zeros((B, LQ, LK), dtype=np.float32)
    for core in range(NCORES):
        b, q0 = core // 4, QS * (core % 4)
        out[b, q0:q0 + QS] = res.results[core]["outp"].astype(np.float32)
        attn[b, q0:q0 + QS] = res.results[core]["attnT"].astype(np.float32).T
    if _trace:
        kernel.last_exec_time_ns = res.exec_time_ns
    return out, attn


# revision 13
# speedup vs baseline: 1.4010x; 1.1745x over previous
"""Distributed Trainium2 (Bass) kernel for additive (Bahdanau) attention.

Strategy
--------
The reference computes  scores[b,i,j] = sum_d v[d] * tanh(qp[b,i,d] + kp[b,j,d])
which is O(B*Lq*Lk*Dk) transcendental work (134M tanh) if done directly.
We replace tanh with a 5-term sine expansion

    tanh(s) ~= sum_m c_m sin(w_m s)            (fit on s in [-7.2, 7.2])

and use  sin(w(a+b)) = sin(wa+f)cos(wb-f) + cos(wa+f)sin(wb-f)  to turn the
score tensor into fp32r matmuls over 128 "trig feature" rows per frequency:

    scoresT = sum_m Qf_m^T @ Pf_m          [Lk, 256] per core

ACT's Sin is only valid on ~[-3.4, 3.4]; arguments are range-reduced on DVE
with a fused magic-number round (one tensor_scalar per pass).  For the lowest
frequency a phase shift f=pi/4 keeps all arguments inside [-0.5, 0.5] turns
(no reduction needed), which lets the PE start the score matmuls early.

Sharding: 8 cores = 2 batches x 4 query-blocks of 256 rows.  K/V (transposed
on host, bf16) are replicated per batch; no collectives.  attn is produced
transposed ([Lk, 256] bf16 per core) and transposed back on the host.
"""

import sys
import numpy as np

if "/opt/trn_rl_repo" not in sys.path:
    sys.path.insert(0, "/opt/trn_rl_repo")

import ml_dtypes

B, LQ, LK, DM, DK, DV = 2, 1024, 1024, 512, 64, 64
QS = 256          # query rows per core
NCORES = 8
M = 5             # sine terms
LN_EPS = 1e-6
MAGIC = 12582912.0  # 1.5 * 2^23: fused (x+MAGIC)-MAGIC == round(x) on DVE

# tanh(s) ~= sum c_m sin(om_m s), weighted LSQ fit on [-7.2, 7.2]
CS = np.array([1.20989, 0.273641, 0.08294, 0.024901, 0.006387])
OMS = np.array([0.365543, 1.109931, 1.887858, 2.738271, 3.786378])
OMP = OMS / (2 * np.pi)

_CACHE = {}

# F_all layout (free-axis offsets, elements): per-freq trig features
#   [Qf1 | Praw1 | Qf2 Qf3 Qf4 Qf5 | Praw2..5]
QO = [0, 1280, 2304, 3328, 4352]          # Q feature slice offsets (len LK)
PO = [1024, 5376, 5632, 5888, 6144]       # raw P sin slice offsets (len QS)
FTOT = 6400
UTOT = 5120   # round-path scratch: [uQ2..5 (4*LK) | uP2..5 (4*QS)]


def _build():
    """Build (and cache) the Bass graph."""
    if "nc" in _CACHE:
        return _CACHE["nc"]
    import concourse.bacc as bacc
    import concourse.mybir as mybir
    from concourse.tile import TileContext

    f32 = mybir.dt.float32
    f32r = mybir.dt.float32r
    bf16 = mybir.dt.bfloat16
    AF = mybir.ActivationFunctionType
    ALU = mybir.AluOpType

    nc = bacc.Bacc("TRN2", target_bir_lowering=False, debug=False,
                   num_devices=NCORES)

    qT_d = nc.dram_tensor("qT", [DM, QS], bf16, kind="ExternalInput").ap()
    qn_d = nc.dram_tensor("qn", [QS, DM], f32, kind="ExternalInput").ap()
    kT_d = nc.dram_tensor("kT", [DM, LK], bf16, kind="ExternalInput").ap()
    vT_d = nc.dram_tensor("vT", [DM, LK], bf16, kind="ExternalInput").ap()
    wq2_d = nc.dram_tensor("wq2", [DM, 128], bf16, kind="ExternalInput").ap()
    wk2_d = nc.dram_tensor("wk2", [DM, 128], bf16, kind="ExternalInput").ap()
    wv_d = nc.dram_tensor("wv", [DM, DV], bf16, kind="ExternalInput").ap()
    wfc_d = nc.dram_tensor("wfc", [DV, DM], f32, kind="ExternalInput").ap()
    pco_d = nc.dram_tensor("pco", [128, M], f32, kind="ExternalInput").ap()
    dvs_d = nc.dram_tensor("dvs", [128, 2], f32, kind="ExternalInput").ap()
    gam_d = nc.dram_tensor("gam", [1, DM], f32, kind="ExternalInput").ap()
    bet_d = nc.dram_tensor("bet", [1, DM], f32, kind="ExternalInput").ap()
    eye_d = nc.dram_tensor("eye", [64, 64], f32, kind="ExternalInput").ap()

    attn_d = nc.dram_tensor("attnT", [LK, QS], bf16, kind="ExternalOutput").ap()
    out_d = nc.dram_tensor("outp", [QS, DM], bf16, kind="ExternalOutput").ap()

    with TileContext(nc) as tc:
        import contextlib
        with contextlib.ExitStack() as ctx:
            const = ctx.enter_context(tc.tile_pool(name="const", bufs=1))
            work = ctx.enter_context(tc.tile_pool(name="work", bufs=1))
            small = ctx.enter_context(tc.tile_pool(name="small", bufs=2))
            ps1 = ctx.enter_context(tc.tile_pool(name="ps1", bufs=1, space="PSUM"))

            # ---------------- inputs -> SBUF (small/critical first) ------
            pco_sb = const.tile([128, M], f32)
            nc.sync.dma_start(out=pco_sb, in_=pco_d)
            dvs_sb = const.tile([128, 2], f32)   # col 0: f1 delta rows
            nc.sync.dma_start(out=dvs_sb, in_=dvs_d)
            eye_sb = const.tile([64, 64], f32)
            nc.sync.dma_start(out=eye_sb, in_=eye_d)
            wk2_sb = const.tile([128, 4, 128], bf16)
            nc.sync.dma_start(out=wk2_sb, in_=wk2_d.rearrange("(t p) m -> p t m", p=128))
            wq2_sb = const.tile([128, 4, 128], bf16)
            nc.sync.dma_start(out=wq2_sb, in_=wq2_d.rearrange("(t p) m -> p t m", p=128))
            wv_sb = const.tile([128, 4, DV], bf16)
            nc.sync.dma_start(out=wv_sb, in_=wv_d.rearrange("(t p) m -> p t m", p=128))
            wfc_f = const.tile([64, DM], f32)
            nc.sync.dma_start(out=wfc_f, in_=wfc_d)
            qT_sb = const.tile([128, 4, QS], bf16)
            nc.sync.dma_start(out=qT_sb, in_=qT_d.rearrange("(t p) l -> p t l", p=128))
            kT_sb = const.tile([128, 4, LK], bf16)
            nc.sync.dma_start(out=kT_sb, in_=kT_d.rearrange("(t p) l -> p t l", p=128))
            vT_sb = const.tile([128, 4, LK], bf16)
            nc.sync.dma_start(out=vT_sb, in_=vT_d.rearrange("(t p) l -> p t l", p=128))
            qn_sb = const.tile([128, 2, DM], f32)
            nc.sync.dma_start(out=qn_sb, in_=qn_d.rearrange("(t p) l -> p t l", p=128))
            gam_sb = const.tile([128, DM], f32)
            gam_b = type(gam_d)(tensor=gam_d.tensor, offset=gam_d.offset,
                                ap=[[0, 128]] + gam_d.ap[1:])
            nc.sync.dma_start(out=gam_sb, in_=gam_b)
            bet_sb = const.tile([128, DM], f32)
            bet_b = type(bet_d)(tensor=bet_d.tensor, offset=bet_d.offset,
                                ap=[[0, 128]] + bet_d.ap[1:])
            nc.sync.dma_start(out=bet_sb, in_=bet_b)

            wfc_r = const.tile([64, DM], f32r)
            nc.vector.tensor_copy(wfc_r[:], wfc_f[:])
            dvQ = const.tile([128, 1], f32)   # f2..5 Q: cos-rows 0.25, sin 0
            nc.vector.memset(dvQ[0:64, :], 0.25)
            nc.vector.memset(dvQ[64:128, :], 0.0)
            dvP = const.tile([128, 1], f32)   # f2..5 P: sin-rows 0, cos 0.25
            nc.vector.memset(dvP[0:64, :], 0.0)
            nc.vector.memset(dvP[64:128, :], 0.25)
            ones_f = const.tile([128, 1], f32)
            nc.vector.memset(ones_f, 1.0)
            ones_r = const.tile([128, 1], f32r)
            nc.vector.tensor_copy(ones_r[:], ones_f[:])
            ones1_f = const.tile([1, 128], f32)
            nc.vector.memset(ones1_f, 1.0)
            ones1_r = const.tile([1, 128], f32r)
            nc.vector.tensor_copy(ones1_r[:], ones1_f[:])
            eps_sb = const.tile([128, 1], f32)
            nc.vector.memset(eps_sb, LN_EPS)

            # ---------------- projections (bf16 matmuls) ----------------
            kp2_ps = ps1.tile([128, 2048], f32, tag="kq")   # 4 banks (reused)
            for t in range(4):
                for h in range(2):
                    nc.tensor.matmul(kp2_ps[:, h * 512:(h + 1) * 512],
                                     wk2_sb[:, t, :],
                                     kT_sb[:, t, h * 512:(h + 1) * 512],
                                     start=(t == 0), stop=(t == 3))
            qp2_ps = ps1.tile([128, QS], f32, tag="qtb")
            for t in range(4):
                nc.tensor.matmul(qp2_ps[:], wq2_sb[:, t, :], qT_sb[:, t, :],
                                 start=(t == 0), stop=(t == 3))
            # ---------------- trig features -----------------------------
            rarg = work.tile([128, FTOT], f32)    # pre-sin arguments
            fall = work.tile([128, FTOT], f32r)   # sin outputs (features)
            pfin = work.tile([128, M, QS], f32r)  # coef-scaled P features
            uu = work.tile([128, UTOT], f32)
            nn = work.tile([128, UTOT], f32)
            UP = 4 * LK                           # P-side offset in uu/nn

            # ---- P side first (runs while kT/vT still stream in) -------
            qp2_sb = work.tile([128, QS], f32)
            nc.vector.tensor_copy(qp2_sb[:], qp2_ps[:])
            nc.vector.tensor_scalar(out=rarg[:, PO[0]:PO[0] + QS], in0=qp2_sb[:],
                                    scalar1=float(OMP[0]), scalar2=dvs_sb[:, 0:1],
                                    op0=ALU.mult, op1=ALU.add)
            nc.scalar.activation(out=fall[:, PO[0]:PO[0] + QS],
                                 in_=rarg[:, PO[0]:PO[0] + QS],
                                 func=AF.Sin, scale=float(2 * np.pi))
            nc.vector.tensor_scalar(out=pfin[:, 0, :],
                                    in0=fall[:, PO[0]:PO[0] + QS],
                                    scalar1=pco_sb[:, 0:1], scalar2=None,
                                    op0=ALU.mult)
            for i, m in enumerate(range(1, M)):
                nc.vector.tensor_scalar(
                    out=uu[:, UP + i * QS:UP + (i + 1) * QS],
                    in0=qp2_sb[:],
                    scalar1=float(OMP[m]), scalar2=dvP[:],
                    op0=ALU.mult, op1=ALU.add)
            nc.vector.tensor_scalar(out=nn[:, UP:UP + 4 * QS],
                                    in0=uu[:, UP:UP + 4 * QS], scalar1=MAGIC,
                                    scalar2=-MAGIC, op0=ALU.add, op1=ALU.add)
            nc.vector.tensor_tensor(out=rarg[:, PO[1]:PO[1] + 4 * QS],
                                    in0=uu[:, UP:UP + 4 * QS],
                                    in1=nn[:, UP:UP + 4 * QS], op=ALU.subtract)
            nc.scalar.activation(out=fall[:, PO[1]:PO[1] + 4 * QS],
                                 in_=rarg[:, PO[1]:PO[1] + 4 * QS],
                                 func=AF.Sin, scale=float(2 * np.pi))
            for m in range(1, M):
                nc.vector.tensor_scalar(out=pfin[:, m, :],
                                        in0=fall[:, PO[m]:PO[m] + QS],
                                        scalar1=pco_sb[:, m:m + 1], scalar2=None,
                                        op0=ALU.mult)

            # ---- Q side: freq-1 direct, freqs 2..5 pipelined chains ----
            kp2_sb = work.tile([128, LK], f32)
            nc.vector.tensor_copy(kp2_sb[:], kp2_ps[:, 0:LK])
            nc.vector.tensor_scalar(out=rarg[:, QO[0]:QO[0] + LK],
                                    in0=kp2_sb[:],
                                    scalar1=float(OMP[0]), scalar2=dvs_sb[:, 0:1],
                                    op0=ALU.mult, op1=ALU.add)
            nc.scalar.activation(out=fall[:, QO[0]:QO[0] + LK],
                                 in_=rarg[:, QO[0]:QO[0] + LK],
                                 func=AF.Sin, scale=float(2 * np.pi))
            for i, m in enumerate(range(1, M)):
                nc.vector.tensor_scalar(out=uu[:, i * LK:(i + 1) * LK],
                                        in0=kp2_sb[:],
                                        scalar1=float(OMP[m]), scalar2=dvQ[:],
                                        op0=ALU.mult, op1=ALU.add)
                nc.vector.tensor_scalar(out=nn[:, i * LK:(i + 1) * LK],
                                        in0=uu[:, i * LK:(i + 1) * LK],
                                        scalar1=MAGIC, scalar2=-MAGIC,
                                        op0=ALU.add, op1=ALU.add)
                nc.vector.tensor_tensor(out=rarg[:, QO[m]:QO[m] + LK],
                                        in0=uu[:, i * LK:(i + 1) * LK],
                                        in1=nn[:, i * LK:(i + 1) * LK],
                                        op=ALU.subtract)
                nc.scalar.activation(out=fall[:, QO[m]:QO[m] + LK],
                                     in_=rarg[:, QO[m]:QO[m] + LK],
                                     func=AF.Sin, scale=float(2 * np.pi))

            # ---------------- scores ------------------------------------
            sc_ps = ps1.tile([128, 2048], f32, tag="kq")   # 8 chunks of 256
            for kc in range(8):
                for m in range(M):
                    nc.tensor.matmul(
                        sc_ps[:, kc * QS:(kc + 1) * QS],
                        fall[:, QO[m] + kc * 128:QO[m] + (kc + 1) * 128],
                        pfin[:, m, :],
                        start=(m == 0), stop=(m == M - 1))

            # vp with keys on partitions: lhsT = vT chunks (after scores)
            vpc_ps = ps1.tile([128, 8, DV], f32, tag="qtb")
            for kc in range(8):
                for t in range(4):
                    nc.tensor.matmul(vpc_ps[:, kc, :],
                                     vT_sb[:, t, kc * 128:(kc + 1) * 128],
                                     wv_sb[:, t, :],
                                     start=(t == 0), stop=(t == 3))
            vp_sb = work.tile([128, 8, DV], f32r)
            nc.vector.tensor_copy(vp_sb[:], vpc_ps[:])

            # ---------------- softmax / av -------------------------------
            du_ps = ps1.tile([64, QS], f32, tag="du")   # attn @ vp accumulator
            den_ps = ps1.tile([1, QS], f32, tag="qtb")  # softmax denominators
            expT = work.tile([128, 8, QS], f32r)
            for kp_ in range(4):  # exp over kc pairs
                nc.scalar.activation(out=expT[:, 2 * kp_:2 * kp_ + 2, :],
                                     in_=sc_ps[:, 2 * kp_ * QS:(2 * kp_ + 2) * QS],
                                     func=AF.Exp, scale=1.0)
            for kc in range(8):
                nc.tensor.matmul(den_ps[:], ones_r[:], expT[:, kc, :],
                                 start=(kc == 0), stop=(kc == 7))
            for kc in range(8):
                nc.tensor.matmul(du_ps[:], vp_sb[:, kc, :], expT[:, kc, :],
                                 start=(kc == 0), stop=(kc == 7))

            rec_f = small.tile([1, QS], f32)
            nc.vector.reciprocal(out=rec_f[:], in_=den_ps[:])
            rec_r = small.tile([1, QS], f32r)
            nc.vector.tensor_copy(rec_r[:], rec_f[:])
            bc_ps = ps1.tile([128, QS], f32, tag="qtb")
            nc.tensor.matmul(bc_ps[:], ones1_r[:], rec_r[:], start=True, stop=True)
            bc_sb = work.tile([128, QS], f32)
            nc.vector.tensor_copy(bc_sb[:], bc_ps[:])

            # attn output (normalized, bf16, transposed)
            for kc in range(8):
                at_sb = small.tile([128, QS], bf16, tag="at")
                nc.vector.tensor_tensor(out=at_sb[:],
                                  in0=expT[:, kc, :].bitcast(f32),
                                  in1=bc_sb[:], op=ALU.mult)
                nc.sync.dma_start(out=attn_d[kc * 128:(kc + 1) * 128, :],
                                  in_=at_sb[:])

            avn = work.tile([64, QS], f32r)
            nc.vector.tensor_tensor(out=avn[:], in0=du_ps[:],
                                    in1=bc_sb[0:64, :], op=ALU.mult)

            # ---------------- fc + residual + layernorm -----------------
            for qc in range(2):
                o_ps = ps1.tile([128, DM], f32, tag="vpo")
                nc.tensor.matmul(o_ps[:], avn[:, qc * 128:(qc + 1) * 128],
                                 wfc_r[:], start=True, stop=True)
                o1 = small.tile([128, DM], f32, tag="o1")
                nc.vector.tensor_tensor(out=o1[:], in0=o_ps[:],
                                        in1=qn_sb[:, qc, :], op=ALU.add)
                st = small.tile([128, 6], f32, tag="st")
                nc.vector.bn_stats(out=st[:], in_=o1[:])
                mv = small.tile([128, 2], f32, tag="mv")
                nc.vector.bn_aggr(out=mv[:], in_=st[:])
                sq = small.tile([128, 1], f32, tag="sq")
                nc.scalar.activation(out=sq[:], in_=mv[:, 1:2], func=AF.Sqrt,
                                     scale=1.0, bias=eps_sb[:])
                rs = small.tile([128, 1], f32, tag="rs")
                nc.vector.reciprocal(out=rs[:], in_=sq[:])
                o2 = small.tile([128, DM], f32, tag="o2")
                nc.vector.tensor_scalar(out=o2[:], in0=o1[:],
                                        scalar1=mv[:, 0:1], scalar2=rs[:],
                                        op0=ALU.subtract, op1=ALU.mult)
                o3 = small.tile([128, DM], f32, tag="o3")
                nc.vector.tensor_tensor(out=o3[:], in0=o2[:], in1=gam_sb[:],
                                        op=ALU.mult)
                o4 = small.tile([128, DM], bf16, tag="o4")
                nc.vector.tensor_tensor(out=o4[:], in0=o3[:], in1=bet_sb[:],
                                        op=ALU.add)
                nc.sync.dma_start(
                    out=out_d.rearrange("(t p) l -> p t l", p=128)[:, qc, :],
                    in_=o4[:])

    nc.finalize()
    _CACHE["nc"] = nc
    return nc


def _prep_inputs(q, k, v, Wq, Wk, Wv, v_param, Wfc, ln_gamma, ln_beta):
    bf = ml_dtypes.bfloat16
    wq2 = np.ascontiguousarray(np.concatenate([Wq, Wq], axis=1)).astype(bf)
    wk2 = np.ascontiguousarray(np.concatenate([Wk, Wk], axis=1)).astype(bf)
    wv = np.ascontiguousarray(Wv).astype(bf)
    wfc = np.ascontiguousarray(Wfc).astype(np.float32)
    pco = np.zeros((128, M), dtype=np.float32)
    for m in range(M):
        s = -1.0 if m == 0 else 1.0   # f1 sin-rows pair with Q's "-cos" rows
        pco[0:64, m] = (s * CS[m] * v_param).astype(np.float32)
        pco[64:128, m] = (CS[m] * v_param).astype(np.float32)
    dvs = np.zeros((128, 2), dtype=np.float32)
    dvs[0:64, 0] = -0.125   # f1: Q "-cos" rows / P sin rows
    dvs[64:128, 0] = 0.125  # f1: Q sin rows / P cos rows
    gam = np.ascontiguousarray(ln_gamma.reshape(1, DM)).astype(np.float32)
    bet = np.ascontiguousarray(ln_beta.reshape(1, DM)).astype(np.float32)
    eye = np.eye(64, dtype=np.float32)

    kT = [np.ascontiguousarray(k[b].T).astype(bf) for b in range(B)]
    vT = [np.ascontiguousarray(v[b].T).astype(bf) for b in range(B)]

    in_maps = []
    for core in range(NCORES):
        b, q0 = core // 4, QS * (core % 4)
        qs = q[b, q0:q0 + QS]
        in_maps.append({
            "qT": np.ascontiguousarray(qs.T).astype(bf),
            "qn": np.ascontiguousarray(qs).astype(np.float32),
            "kT": kT[b],
            "vT": vT[b],
            "wq2": wq2, "wk2": wk2, "wv": wv, "wfc": wfc,
            "pco": pco, "dvs": dvs, "gam": gam, "bet": bet, "eye": eye,
        })
    return in_maps


def kernel(q, k, v, Wq, Wk, Wv, v_param, Wfc, ln_gamma, ln_beta,
           _trace=False):
    q = np.asarray(q); k = np.asarray(k); v = np.asarray(v)
    Wq = np.asarray(Wq); Wk = np.asarray(Wk); Wv = np.asarray(Wv)
    v_param = np.asarray(v_param); Wfc = np.asarray(Wfc)
    ln_gamma = np.asarray(ln_gamma); ln_beta = np.asarray(ln_beta)

    from concourse.bass_utils import run_bass_kernel_spmd

    nc = _build()
    in_maps = _prep_inputs(q, k, v, Wq, Wk, Wv, v_param, Wfc, ln_gamma, ln_beta)
    res = run_bass_kernel_spmd(nc, in_maps, core_ids=list(range(NCORES)),
                               trace=_trace)

    out = np.zeros((B, LQ, DM), dtype=np.float32)
    attn = np.zeros((B, LQ, LK), dtype=np.float32)
    for core in range(NCORES):
        b, q0 = core // 4, QS * (core % 4)
        out[b, q0:q0 + QS] = res.results[core]["outp"].astype(np.float32)
        attn[b, q0:q0 + QS] = res.results[core]["attnT"].astype(np.float32).T
    if _trace:
        kernel.last_exec_time_ns = res.exec_time_ns
    return out, attn
